# revision 23
# baseline (speedup 1.0000x reference)
"""Trainium2 Bass kernel for nn_CoreAttention (causal attention).

Problem (hardcoded): Q/K/V [SQ=2048, B=2, H=16, D=64] fp32, causal mask,
softmax(QK^T/8) @ V, output [2048, 2, 1024].

Sharding: batch*heads (32) split 4 heads per core across 8 cores.

Per-core device layout (host prepares these in the shard step):
  qt  [256, 2048] bf16 : Q^T d-major; row = pair*128 + head_local*64 + d
  kt  [256, 2048] bf16 : K^T same layout
  v   [4, 2048, 65]    : V natural per head + ones column (denominator)
  out [4, 65, 2048] f32: rows 0-63 unnormalized context^T, row 64 the
                         softmax denominator; host divides + transposes.

Algorithm per head-pair (2 heads packed on 128 SBUF partitions):
  For each q-block j (512 wide), software-pipelined over k-blocks i
  (128 wide, causally trimmed):
    BMM1  S^T[k, q] for both heads via PE row-tiled pair -> PSUM [128,2,512]
    exp   ScalarE activation (scale=1/8) PSUM -> bf16 P in SBUF
    mask  diagonal 128x128 sub-block zeroed on GpSimd (affine_select)
          after exp (keeps mask off the BMM1->exp critical path)
    BMM2  ctx^T[65, q] += V'.T @ P^T per head (V' has ones column; row 64
          accumulates the denominator), issued one i behind BMM1 so the
          PE never waits on the ACT engine.
  Epilogue: DVE copy ctx PSUM->SBUF, DMA out (no on-device normalize).
"""

import sys

sys.path.insert(0, "/opt/trn_rl_repo")

import numpy as np

from contextlib import ExitStack

import concourse.bass as bass
import concourse.mybir as mybir
import concourse.tile as tile
from concourse import bacc

SQ, B, H, D = 2048, 2, 16, 64
NCORES = 8
HPC = 4  # heads per core
NPAIR = 2  # head pairs per core
KB = 128  # k block
QB = 512  # q block
NKB = SQ // KB  # 16
NQB = SQ // QB  # 4
NORM = 8.0  # sqrt(D) * layer_number

F32 = mybir.dt.float32
BF16 = mybir.dt.bfloat16
I16 = mybir.dt.int16

# Schraudolph-style exp approximation, emitted directly as bf16 bit
# patterns: bf16(exp(s/8)) ~ bitcast(int16(round(s * SCH_A + SCH_B))).
# bf16 has an 8-bit exponent (bias 127) and 7-bit mantissa, so
# SCH_A = 2^7 / (8 * ln 2), SCH_B = 127 * 2^7 - C with C tuned to
# center the sawtooth error (~1.8% rms per element).
SCH_A = 128.0 / (8.0 * np.log(2.0))
SCH_B = 127.0 * 128.0 - 4.8


def build_attention(nc, tc, ctx_stack):
    qt = nc.dram_tensor("qt", [NPAIR * 128, SQ], BF16, kind="ExternalInput").ap()
    kt = nc.dram_tensor("kt", [NPAIR * 128, SQ], BF16, kind="ExternalInput").ap()
    # v carries a host-prepared ones column at d=D (softmax denominator
    # trick) and is pre-swizzled to the SBUF layout [128, HPC, NKB, 65].
    v = nc.dram_tensor(
        "v", [128, HPC * NKB * (D + 1)], BF16, kind="ExternalInput"
    ).ap()
    out = nc.dram_tensor("out", [HPC, D + 1, SQ], F32, kind="ExternalOutput").ap()

    ec = ctx_stack.enter_context
    consts = ec(tc.tile_pool(name="consts", bufs=1))
    inp = ec(tc.tile_pool(name="inp", bufs=1))
    pp = ec(tc.tile_pool(name="pp", bufs=5))
    ep = ec(tc.tile_pool(name="ep", bufs=4))
    psum_s = ec(tc.tile_pool(name="psum_s", bufs=3, space="PSUM"))
    psum_c = ec(tc.tile_pool(name="psum_c", bufs=1, space="PSUM"))

    # Resident inputs.
    qt_sb = inp.tile([128, NPAIR, SQ], BF16)
    kt_sb = inp.tile([128, NPAIR, SQ], BF16)
    vp_sb = inp.tile([128, HPC, NKB, D + 1], BF16)

    # Chunked input loads, ordered by first use (j runs descending, k
    # ascending): kt chunks ascending, qt chunks descending, vp ascending.
    # v arrives host-pre-swizzled to the SBUF layout so each DMA chunk is
    # partition-linear.
    qt_r = qt.rearrange("(pr p) q -> p pr q", p=128)
    kt_r = kt.rearrange("(pr p) q -> p pr q", p=128)
    v_r = v.rearrange("p (g n d) -> p g n d", g=HPC, n=NKB)
    # Pair-major issue order: pair 1's inputs are not needed until the
    # second half of the kernel, so all of pair 0's chunks go first.
    for pr in range(NPAIR):
        for c in range(NQB):
            ksl = slice(c * QB, (c + 1) * QB)
            qsl = slice((NQB - 1 - c) * QB, (NQB - c) * QB)
            bl = slice(4 * c, 4 * c + 4)
            nc.sync.dma_start(out=kt_sb[:, pr, ksl], in_=kt_r[:, pr, ksl])
            nc.sync.dma_start(out=qt_sb[:, pr, qsl], in_=qt_r[:, pr, qsl])
            for g in (2 * pr, 2 * pr + 1):
                nc.sync.dma_start(out=vp_sb[:, g, bl, :], in_=v_r[:, g, bl, :])

    def mm1_pair(pr, j, i, s_ps):
        """BMM1: S^T[k, q] for both heads, row-tiled on the PE."""
        t = i - 4 * j
        qs = max(0, 128 * t)
        nc.tensor.matmul(
            s_ps[:, 0, qs:QB],
            lhsT=kt_sb[0:64, pr, i * KB : (i + 1) * KB],
            rhs=qt_sb[0:64, pr, j * QB + qs : (j + 1) * QB],
            start=True,
            stop=True,
            tile_position=(0, 0),
        )
        nc.tensor.matmul(
            s_ps[:, 1, qs:QB],
            lhsT=kt_sb[64:128, pr, i * KB : (i + 1) * KB],
            rhs=qt_sb[64:128, pr, j * QB + qs : (j + 1) * QB],
            start=True,
            stop=True,
            tile_position=(64, 0),
        )

    def mm2_pair(pr, j, i, n_i, p_sb, qs, ctx_A, ctx_B):
        """BMM2: ctx^T[0:64] += V.T @ P^T ; row 64 accumulates sums."""
        nc.tensor.matmul(
            ctx_A[:, qs:QB],
            lhsT=vp_sb[:, 2 * pr, i, :],
            rhs=p_sb[:, 0, qs:QB],
            start=(i == 0),
            stop=(i == n_i - 1),
        )
        nc.tensor.matmul(
            ctx_B[:, qs:QB],
            lhsT=vp_sb[:, 2 * pr + 1, i, :],
            rhs=p_sb[:, 1, qs:QB],
            start=(i == 0),
            stop=(i == n_i - 1),
        )

    # Flat slot list across all (pair, j, i) so the software pipeline runs
    # through j/pair boundaries without a bubble. j descending (longest
    # i-loops first, so the kernel tail is short).
    slots = []
    for pr in range(NPAIR):
        for j in range(NQB - 1, -1, -1):
            n_i = 4 * j + 4  # causal: k blocks 0 .. 4j+3
            for i in range(n_i):
                slots.append((pr, j, i, n_i))

    pending = []  # queue of (pr, j, i, n_i, p_sb, qs, ctx_A, ctx_B)
    ctxs = {}

    def drain_one():
        ent = pending.pop(0)
        mm2_pair(*ent)
        if ent[2] == ent[3] - 1:  # finished a (pair, j): epilogue
            ppr, pj = ent[0], ent[1]
            # Split the two PSUM->SBUF copies across DVE and the slack
            # ScalarE so neither blocks its engine's pipeline at the
            # j boundary.
            for g, ctx, eng in (
                (2 * ppr, ent[6], "v"),
                (2 * ppr + 1, ent[7], "s"),
            ):
                ctxn = ep.tile([D + 1, QB], F32, tag="ctxn")
                if eng == "v":
                    nc.vector.tensor_copy(ctxn, ctx)
                else:
                    nc.scalar.copy(ctxn, ctx)
                nc.sync.dma_start(
                    out=out[g, :, pj * QB : (pj + 1) * QB], in_=ctxn
                )

    n_offd = 0
    for pr, j, i, n_i in slots:
        if i == 0:
            ctxs[(pr, j)] = (
                psum_c.tile([D + 1, QB], F32, tag="ctxA", name="ctxA"),
                psum_c.tile([D + 1, QB], F32, tag="ctxB", name="ctxB"),
            )
        ctx_A, ctx_B = ctxs[(pr, j)]
        t = i - 4 * j
        qs = max(0, 128 * t)  # q start within the 512 block
        s_ps = psum_s.tile([128, 2, QB], F32, tag="s")
        mm1_pair(pr, j, i, s_ps)
        # Lag-2 software pipeline; at each j's first two slots skip the
        # drain so the previous j's diagonal-tail BMM2s (whose P arrives
        # late via the exp+mask chain) drop off the critical path.
        if i >= 2:
            while len(pending) > 2:
                drain_one()
        if t < 0 and (n_offd := n_offd + 1) % 2:
            # Offload this slot's exp to DVE via the int16-Schraudolph
            # trick: bf16(exp(s/8)) == bitcast(int16(s * SCH_A + SCH_B)).
            p_i16 = pp.tile([128, 2, QB], I16, tag="p")
            nc.vector.tensor_scalar(
                out=p_i16[:, :, qs:QB],
                in0=s_ps[:, :, qs:QB],
                scalar1=SCH_A,
                scalar2=SCH_B,
                op0=mybir.AluOpType.mult,
                op1=mybir.AluOpType.add,
            )
            p_sb = p_i16.bitcast(BF16)
        else:
            p_sb = pp.tile([128, 2, QB], BF16, tag="p")
            nc.scalar.activation(
                p_sb[:, :, qs:QB],
                s_ps[:, :, qs:QB],
                mybir.ActivationFunctionType.Exp,
                scale=1.0 / NORM,
            )
            if t >= 0:
                # Diagonal sub-block: zero the strict upper triangle
                # (q < k) of P for both heads, post-exp, on GpSimd.
                nc.gpsimd.affine_select(
                    out=p_sb[:, :, qs : qs + 128],
                    in_=p_sb[:, :, qs : qs + 128],
                    compare_op=mybir.AluOpType.is_ge,
                    fill=0.0,
                    base=0,
                    pattern=[[0, 2], [1, 128]],  # iota over q, ignore head
                    channel_multiplier=-1,  # -k per partition
                )
        pending.append((pr, j, i, n_i, p_sb, qs, ctx_A, ctx_B))
    while pending:
        drain_one()


def _build_nc():
    nc = bacc.Bacc(
        "TRN2", target_bir_lowering=False, debug=False, num_devices=NCORES
    )
    with tile.TileContext(nc) as tc, ExitStack() as ctx_stack:
        build_attention(nc, tc, ctx_stack)
    nc.compile()
    return nc


_NC_CACHE = {}


def get_nc():
    if "nc" not in _NC_CACHE:
        _NC_CACHE["nc"] = _build_nc()
    return _NC_CACHE["nc"]


def shard_inputs(query_layer, key_layer, value_layer):
    """Full [SQ, B, H, D] fp32 inputs -> list of 8 per-core input dicts."""
    import ml_dtypes

    bf16 = np.dtype(ml_dtypes.bfloat16)
    q = np.asarray(query_layer, dtype=np.float32)
    k = np.asarray(key_layer, dtype=np.float32)
    v = np.asarray(value_layer, dtype=np.float32)
    # [SQ, B, H, D] -> [B*H, D, SQ] (d-major) for Q/K; [B*H, SQ, D] for V.
    qt = np.ascontiguousarray(q.transpose(1, 2, 3, 0).reshape(B * H, D, SQ))
    kt = np.ascontiguousarray(k.transpose(1, 2, 3, 0).reshape(B * H, D, SQ))
    vn = v.transpose(1, 2, 0, 3).reshape(B * H, SQ, D)
    vn = np.concatenate([vn, np.ones((B * H, SQ, 1), np.float32)], axis=2)
    qt = qt.astype(bf16)
    kt = kt.astype(bf16)
    vn = np.ascontiguousarray(vn.astype(bf16))
    in_maps = []
    for c in range(NCORES):
        sl = slice(HPC * c, HPC * (c + 1))
        # Swizzle v to the device SBUF layout [128, HPC, NKB, 65].
        vc = vn[sl].reshape(HPC, NKB, 128, D + 1).transpose(2, 0, 1, 3)
        in_maps.append(
            {
                "qt": np.ascontiguousarray(qt[sl].reshape(HPC * D, SQ)),
                "kt": np.ascontiguousarray(kt[sl].reshape(HPC * D, SQ)),
                "v": np.ascontiguousarray(vc.reshape(128, HPC * NKB * (D + 1))),
            }
        )
    return in_maps


def gather_outputs(results):
    """8 per-core {'out': [4, 65, 2048]} -> full [SQ, B, H*D] fp32."""
    raw = np.stack([np.asarray(results[c]["out"], dtype=np.float32) for c in range(NCORES)])
    raw = raw.reshape(B * H, D + 1, SQ)
    ctx_t = raw[:, :D, :] / raw[:, D : D + 1, :]  # normalize by denominator
    full = ctx_t.transpose(2, 0, 1).reshape(SQ, B, H * D)
    return np.ascontiguousarray(full.astype(np.float32))


def run_on_device(in_maps, trace=False):
    from concourse.bass_utils import run_bass_kernel_spmd

    nc = get_nc()
    res = run_bass_kernel_spmd(
        nc, in_maps, core_ids=list(range(NCORES)), trace=trace
    )
    return res


def kernel(query_layer, key_layer, value_layer, attention_mask=None):
    in_maps = shard_inputs(query_layer, key_layer, value_layer)
    res = run_on_device(in_maps, trace=False)
    return gather_outputs(res.results)


# revision 24
# speedup vs baseline: 1.1782x; 1.1782x over previous
"""Trainium2 Bass kernel for nn_CoreAttention (causal attention).

Problem (hardcoded): Q/K/V [SQ=2048, B=2, H=16, D=64] fp32, causal mask,
softmax(QK^T/8) @ V, output [2048, 2, 1024].

Sharding: batch*heads (32) split 4 heads per core across 8 cores.

Per-core device layout (host prepares these in the shard step):
  qt  [256, 2048] bf16 : Q^T d-major; row = pair*128 + head_local*64 + d
  kt  [256, 2048] bf16 : K^T same layout
  v   [4, 2048, 65]    : V natural per head + ones column (denominator)
  out [4, 65, 2048] f32: rows 0-63 unnormalized context^T, row 64 the
                         softmax denominator; host divides + transposes.

Algorithm per head-pair (2 heads packed on 128 SBUF partitions):
  For each q-block j (512 wide), software-pipelined over k-blocks i
  (128 wide, causally trimmed):
    BMM1  S^T[k, q] for both heads via PE row-tiled pair -> PSUM [128,2,512]
    exp   ScalarE activation (scale=1/8) PSUM -> bf16 P in SBUF
    mask  diagonal 128x128 sub-block zeroed on GpSimd (affine_select)
          after exp (keeps mask off the BMM1->exp critical path)
    BMM2  ctx^T[65, q] += V'.T @ P^T per head (V' has ones column; row 64
          accumulates the denominator), issued one i behind BMM1 so the
          PE never waits on the ACT engine.
  Epilogue: DVE copy ctx PSUM->SBUF, DMA out (no on-device normalize).
"""

import sys

sys.path.insert(0, "/opt/trn_rl_repo")

import numpy as np

from contextlib import ExitStack

import concourse.bass as bass
import concourse.mybir as mybir
import concourse.tile as tile
from concourse import bacc

SQ, B, H, D = 2048, 2, 16, 64
NCORES = 8
HPC = 4  # heads per core
NPAIR = 2  # head pairs per core
KB = 128  # k block
QB = 512  # q block
NKB = SQ // KB  # 16
NQB = SQ // QB  # 4
NORM = 8.0  # sqrt(D) * layer_number

F32 = mybir.dt.float32
BF16 = mybir.dt.bfloat16
I16 = mybir.dt.int16

# Schraudolph-style exp approximation, emitted directly as bf16 bit
# patterns: bf16(exp(s/8)) ~ bitcast(int16(round(s * SCH_A + SCH_B))).
# bf16 has an 8-bit exponent (bias 127) and 7-bit mantissa, so
# SCH_A = 2^7 / (8 * ln 2), SCH_B = 127 * 2^7 - C with C tuned to
# center the sawtooth error (~1.8% rms per element).
SCH_A = 128.0 / (8.0 * np.log(2.0))
SCH_B = 127.0 * 128.0 - 4.8


def build_attention(nc, tc, ctx_stack):
    qt = nc.dram_tensor("qt", [NPAIR * 128, SQ], BF16, kind="ExternalInput").ap()
    kt = nc.dram_tensor("kt", [NPAIR * 128, SQ], BF16, kind="ExternalInput").ap()
    # v carries a host-prepared ones column at d=D (softmax denominator
    # trick) and is pre-swizzled to the SBUF layout [128, HPC, NKB, 65].
    v = nc.dram_tensor(
        "v", [128, HPC * NKB * (D + 1)], BF16, kind="ExternalInput"
    ).ap()
    out = nc.dram_tensor("out", [HPC, D + 1, SQ], F32, kind="ExternalOutput").ap()

    ec = ctx_stack.enter_context
    consts = ec(tc.tile_pool(name="consts", bufs=1))
    inp = ec(tc.tile_pool(name="inp", bufs=1))
    pp = ec(tc.tile_pool(name="pp", bufs=5))
    ep = ec(tc.tile_pool(name="ep", bufs=4))
    psum_s = ec(tc.tile_pool(name="psum_s", bufs=3, space="PSUM"))
    psum_c = ec(tc.tile_pool(name="psum_c", bufs=1, space="PSUM"))

    # Resident inputs.
    qt_sb = inp.tile([128, NPAIR, SQ], BF16)
    kt_sb = inp.tile([128, NPAIR, SQ], BF16)
    vp_sb = inp.tile([128, HPC, NKB, D + 1], BF16)

    # Chunked input loads, ordered by first use (j runs descending, k
    # ascending): kt chunks ascending, qt chunks descending, vp ascending.
    # v arrives host-pre-swizzled to the SBUF layout so each DMA chunk is
    # partition-linear.
    qt_r = qt.rearrange("(pr p) q -> p pr q", p=128)
    kt_r = kt.rearrange("(pr p) q -> p pr q", p=128)
    v_r = v.rearrange("p (g n d) -> p g n d", g=HPC, n=NKB)
    # Pair-major issue order: pair 1's inputs are not needed until the
    # second half of the kernel, so all of pair 0's chunks go first.
    for pr in range(NPAIR):
        for c in range(NQB):
            ksl = slice(c * QB, (c + 1) * QB)
            qsl = slice((NQB - 1 - c) * QB, (NQB - c) * QB)
            bl = slice(4 * c, 4 * c + 4)
            nc.sync.dma_start(out=kt_sb[:, pr, ksl], in_=kt_r[:, pr, ksl])
            nc.sync.dma_start(out=qt_sb[:, pr, qsl], in_=qt_r[:, pr, qsl])
            for g in (2 * pr, 2 * pr + 1):
                nc.sync.dma_start(out=vp_sb[:, g, bl, :], in_=v_r[:, g, bl, :])

    def mm1_pair(pr, j, i, s_ps):
        """BMM1: S^T[k, q] for both heads, row-tiled on the PE."""
        t = i - 4 * j
        qs = max(0, 128 * t)
        nc.tensor.matmul(
            s_ps[:, 0, qs:QB],
            lhsT=kt_sb[0:64, pr, i * KB : (i + 1) * KB],
            rhs=qt_sb[0:64, pr, j * QB + qs : (j + 1) * QB],
            start=True,
            stop=True,
            tile_position=(0, 0),
        )
        nc.tensor.matmul(
            s_ps[:, 1, qs:QB],
            lhsT=kt_sb[64:128, pr, i * KB : (i + 1) * KB],
            rhs=qt_sb[64:128, pr, j * QB + qs : (j + 1) * QB],
            start=True,
            stop=True,
            tile_position=(64, 0),
        )

    def mm2_pair(pr, j, i, n_i, p_sb, qs, ctx_A, ctx_B):
        """BMM2: ctx^T[0:64] += V.T @ P^T ; row 64 accumulates sums."""
        nc.tensor.matmul(
            ctx_A[:, qs:QB],
            lhsT=vp_sb[:, 2 * pr, i, :],
            rhs=p_sb[:, 0, qs:QB],
            start=(i == 0),
            stop=(i == n_i - 1),
        )
        nc.tensor.matmul(
            ctx_B[:, qs:QB],
            lhsT=vp_sb[:, 2 * pr + 1, i, :],
            rhs=p_sb[:, 1, qs:QB],
            start=(i == 0),
            stop=(i == n_i - 1),
        )

    # Flat slot list across all (pair, j, i) so the software pipeline runs
    # through j/pair boundaries without a bubble. j descending (longest
    # i-loops first, so the kernel tail is short).
    slots = []
    for pr in range(NPAIR):
        for j in range(NQB - 1, -1, -1):
            n_i = 4 * j + 4  # causal: k blocks 0 .. 4j+3
            for i in range(n_i):
                slots.append((pr, j, i, n_i))

    pending = []  # queue of (pr, j, i, n_i, p_sb, qs, ctx_A, ctx_B)
    ctxs = {}

    def drain_one():
        ent = pending.pop(0)
        mm2_pair(*ent)
        if ent[2] == ent[3] - 1:  # finished a (pair, j): epilogue
            ppr, pj = ent[0], ent[1]
            # Split the two PSUM->SBUF copies across DVE and the slack
            # ScalarE so neither blocks its engine's pipeline at the
            # j boundary.
            for g, ctx, eng in (
                (2 * ppr, ent[6], "v"),
                (2 * ppr + 1, ent[7], "s"),
            ):
                ctxn = ep.tile([D + 1, QB], F32, tag="ctxn")
                if eng == "v":
                    nc.vector.tensor_copy(ctxn, ctx)
                else:
                    nc.scalar.copy(ctxn, ctx)
                nc.sync.dma_start(
                    out=out[g, :, pj * QB : (pj + 1) * QB], in_=ctxn
                )

    n_offd = 0
    for pr, j, i, n_i in slots:
        if i == 0:
            ctxs[(pr, j)] = (
                psum_c.tile([D + 1, QB], F32, tag="ctxA", name="ctxA"),
                psum_c.tile([D + 1, QB], F32, tag="ctxB", name="ctxB"),
            )
        ctx_A, ctx_B = ctxs[(pr, j)]
        t = i - 4 * j
        qs = max(0, 128 * t)  # q start within the 512 block
        s_ps = psum_s.tile([128, 2, QB], F32, tag="s")
        mm1_pair(pr, j, i, s_ps)
        # Lag-1 software pipeline; at each j's first two slots skip the
        # drain so the previous j's diagonal-tail BMM2s (whose P arrives
        # late via the exp+mask chain) drop off the critical path.
        if i >= 2:
            while len(pending) > 1:
                drain_one()
        if t < 0 and (n_offd := n_offd + 1) % 2:
            # Offload this slot's exp to DVE via the int16-Schraudolph
            # trick: bf16(exp(s/8)) == bitcast(int16(s * SCH_A + SCH_B)).
            p_i16 = pp.tile([128, 2, QB], I16, tag="p")
            nc.vector.tensor_scalar(
                out=p_i16[:, :, qs:QB],
                in0=s_ps[:, :, qs:QB],
                scalar1=SCH_A,
                scalar2=SCH_B,
                op0=mybir.AluOpType.mult,
                op1=mybir.AluOpType.add,
            )
            p_sb = p_i16.bitcast(BF16)
        else:
            p_sb = pp.tile([128, 2, QB], BF16, tag="p")
            nc.scalar.activation(
                p_sb[:, :, qs:QB],
                s_ps[:, :, qs:QB],
                mybir.ActivationFunctionType.Exp,
                scale=1.0 / NORM,
            )
            if t >= 0:
                # Diagonal sub-block: zero the strict upper triangle
                # (q < k) of P for both heads, post-exp, on GpSimd.
                nc.gpsimd.affine_select(
                    out=p_sb[:, :, qs : qs + 128],
                    in_=p_sb[:, :, qs : qs + 128],
                    compare_op=mybir.AluOpType.is_ge,
                    fill=0.0,
                    base=0,
                    pattern=[[0, 2], [1, 128]],  # iota over q, ignore head
                    channel_multiplier=-1,  # -k per partition
                )
        pending.append((pr, j, i, n_i, p_sb, qs, ctx_A, ctx_B))
    while pending:
        drain_one()


def _build_nc():
    nc = bacc.Bacc(
        "TRN2", target_bir_lowering=False, debug=False, num_devices=NCORES
    )
    with tile.TileContext(nc) as tc, ExitStack() as ctx_stack:
        build_attention(nc, tc, ctx_stack)
    nc.compile()
    return nc


_NC_CACHE = {}


def get_nc():
    if "nc" not in _NC_CACHE:
        _NC_CACHE["nc"] = _build_nc()
    return _NC_CACHE["nc"]


def shard_inputs(query_layer, key_layer, value_layer):
    """Full [SQ, B, H, D] fp32 inputs -> list of 8 per-core input dicts."""
    import ml_dtypes

    bf16 = np.dtype(ml_dtypes.bfloat16)
    q = np.asarray(query_layer, dtype=np.float32)
    k = np.asarray(key_layer, dtype=np.float32)
    v = np.asarray(value_layer, dtype=np.float32)
    # [SQ, B, H, D] -> [B*H, D, SQ] (d-major) for Q/K; [B*H, SQ, D] for V.
    qt = np.ascontiguousarray(q.transpose(1, 2, 3, 0).reshape(B * H, D, SQ))
    kt = np.ascontiguousarray(k.transpose(1, 2, 3, 0).reshape(B * H, D, SQ))
    vn = v.transpose(1, 2, 0, 3).reshape(B * H, SQ, D)
    vn = np.concatenate([vn, np.ones((B * H, SQ, 1), np.float32)], axis=2)
    qt = qt.astype(bf16)
    kt = kt.astype(bf16)
    vn = np.ascontiguousarray(vn.astype(bf16))
    in_maps = []
    for c in range(NCORES):
        sl = slice(HPC * c, HPC * (c + 1))
        # Swizzle v to the device SBUF layout [128, HPC, NKB, 65].
        vc = vn[sl].reshape(HPC, NKB, 128, D + 1).transpose(2, 0, 1, 3)
        in_maps.append(
            {
                "qt": np.ascontiguousarray(qt[sl].reshape(HPC * D, SQ)),
                "kt": np.ascontiguousarray(kt[sl].reshape(HPC * D, SQ)),
                "v": np.ascontiguousarray(vc.reshape(128, HPC * NKB * (D + 1))),
            }
        )
    return in_maps


def gather_outputs(results):
    """8 per-core {'out': [4, 65, 2048]} -> full [SQ, B, H*D] fp32."""
    raw = np.stack([np.asarray(results[c]["out"], dtype=np.float32) for c in range(NCORES)])
    raw = raw.reshape(B * H, D + 1, SQ)
    ctx_t = raw[:, :D, :] / raw[:, D : D + 1, :]  # normalize by denominator
    full = ctx_t.transpose(2, 0, 1).reshape(SQ, B, H * D)
    return np.ascontiguousarray(full.astype(np.float32))


def run_on_device(in_maps, trace=False):
    from concourse.bass_utils import run_bass_kernel_spmd

    nc = get_nc()
    res = run_bass_kernel_spmd(
        nc, in_maps, core_ids=list(range(NCORES)), trace=trace
    )
    return res


def kernel(query_layer, key_layer, value_layer, attention_mask=None):
    in_maps = shard_inputs(query_layer, key_layer, value_layer)
    res = run_on_device(in_maps, trace=False)
    return gather_outputs(res.results)


# revision 26
# speedup vs baseline: 1.1794x; 1.0010x over previous
"""Trainium2 Bass kernel for nn_CoreAttention (causal attention).

Problem (hardcoded): Q/K/V [SQ=2048, B=2, H=16, D=64] fp32, causal mask,
softmax(QK^T/8) @ V, output [2048, 2, 1024].

Sharding: batch*heads (32) split 4 heads per core across 8 cores.

Per-core device layout (host prepares these in the shard step):
  qt  [256, 2048] bf16 : Q^T d-major; row = pair*128 + head_local*64 + d
  kt  [256, 2048] bf16 : K^T same layout
  v   [4, 2048, 65]    : V natural per head + ones column (denominator)
  out [4, 65, 2048] f32: rows 0-63 unnormalized context^T, row 64 the
                         softmax denominator; host divides + transposes.

Algorithm per head-pair (2 heads packed on 128 SBUF partitions):
  For each q-block j (512 wide), software-pipelined over k-blocks i
  (128 wide, causally trimmed):
    BMM1  S^T[k, q] for both heads via PE row-tiled pair -> PSUM [128,2,512]
    exp   ScalarE activation (scale=1/8) PSUM -> bf16 P in SBUF
    mask  diagonal 128x128 sub-block zeroed on GpSimd (affine_select)
          after exp (keeps mask off the BMM1->exp critical path)
    BMM2  ctx^T[65, q] += V'.T @ P^T per head (V' has ones column; row 64
          accumulates the denominator), issued one i behind BMM1 so the
          PE never waits on the ACT engine.
  Epilogue: DVE copy ctx PSUM->SBUF, DMA out (no on-device normalize).
"""

import sys

sys.path.insert(0, "/opt/trn_rl_repo")

import numpy as np

from contextlib import ExitStack

import concourse.bass as bass
import concourse.mybir as mybir
import concourse.tile as tile
from concourse import bacc

SQ, B, H, D = 2048, 2, 16, 64
NCORES = 8
HPC = 4  # heads per core
NPAIR = 2  # head pairs per core
KB = 128  # k block
QB = 512  # q block
NKB = SQ // KB  # 16
NQB = SQ // QB  # 4
NORM = 8.0  # sqrt(D) * layer_number

F32 = mybir.dt.float32
BF16 = mybir.dt.bfloat16
I16 = mybir.dt.int16

# Schraudolph-style exp approximation, emitted directly as bf16 bit
# patterns: bf16(exp(s/8)) ~ bitcast(int16(round(s * SCH_A + SCH_B))).
# bf16 has an 8-bit exponent (bias 127) and 7-bit mantissa, so
# SCH_A = 2^7 / (8 * ln 2), SCH_B = 127 * 2^7 - C with C tuned to
# center the sawtooth error (~1.8% rms per element).
SCH_A = 128.0 / (8.0 * np.log(2.0))
SCH_B = 127.0 * 128.0 - 4.8


def build_attention(nc, tc, ctx_stack):
    qt = nc.dram_tensor("qt", [NPAIR * 128, SQ], BF16, kind="ExternalInput").ap()
    kt = nc.dram_tensor("kt", [NPAIR * 128, SQ], BF16, kind="ExternalInput").ap()
    # v carries a host-prepared ones column at d=D (softmax denominator
    # trick) and is pre-swizzled to the SBUF layout [128, HPC, NKB, 65].
    v = nc.dram_tensor(
        "v", [128, HPC * NKB * (D + 1)], BF16, kind="ExternalInput"
    ).ap()
    out = nc.dram_tensor("out", [HPC, D + 1, SQ], F32, kind="ExternalOutput").ap()

    ec = ctx_stack.enter_context
    consts = ec(tc.tile_pool(name="consts", bufs=1))
    inp = ec(tc.tile_pool(name="inp", bufs=1))
    pp = ec(tc.tile_pool(name="pp", bufs=5))
    ep = ec(tc.tile_pool(name="ep", bufs=4))
    psum_s = ec(tc.tile_pool(name="psum_s", bufs=3, space="PSUM"))
    psum_c = ec(tc.tile_pool(name="psum_c", bufs=1, space="PSUM"))

    # Resident inputs.
    qt_sb = inp.tile([128, NPAIR, SQ], BF16)
    kt_sb = inp.tile([128, NPAIR, SQ], BF16)
    vp_sb = inp.tile([128, HPC, NKB, D + 1], BF16)

    # Chunked input loads, ordered by first use (j runs descending, k
    # ascending): kt chunks ascending, qt chunks descending, vp ascending.
    # v arrives host-pre-swizzled to the SBUF layout so each DMA chunk is
    # partition-linear.
    qt_r = qt.rearrange("(pr p) q -> p pr q", p=128)
    kt_r = kt.rearrange("(pr p) q -> p pr q", p=128)
    v_r = v.rearrange("p (g n d) -> p g n d", g=HPC, n=NKB)
    # Pair-major issue order: pair 1's inputs are not needed until the
    # second half of the kernel, so all of pair 0's chunks go first.
    for pr in range(NPAIR):
        for c in range(NQB):
            ksl = slice(c * QB, (c + 1) * QB)
            qsl = slice((NQB - 1 - c) * QB, (NQB - c) * QB)
            bl = slice(4 * c, 4 * c + 4)
            if pr == 0 and c == 0:
                # Tiny first chunk so the very first BMM1 (needs only
                # k-block 0) starts as early as possible.
                nc.sync.dma_start(out=kt_sb[:, 0, 0:KB], in_=kt_r[:, 0, 0:KB])
                nc.sync.dma_start(out=qt_sb[:, pr, qsl], in_=qt_r[:, pr, qsl])
                nc.sync.dma_start(
                    out=kt_sb[:, 0, KB:QB], in_=kt_r[:, 0, KB:QB]
                )
            else:
                nc.sync.dma_start(out=kt_sb[:, pr, ksl], in_=kt_r[:, pr, ksl])
                nc.sync.dma_start(out=qt_sb[:, pr, qsl], in_=qt_r[:, pr, qsl])
            for g in (2 * pr, 2 * pr + 1):
                nc.sync.dma_start(out=vp_sb[:, g, bl, :], in_=v_r[:, g, bl, :])

    def mm1_pair(pr, j, i, s_ps):
        """BMM1: S^T[k, q] for both heads, row-tiled on the PE."""
        t = i - 4 * j
        qs = max(0, 128 * t)
        nc.tensor.matmul(
            s_ps[:, 0, qs:QB],
            lhsT=kt_sb[0:64, pr, i * KB : (i + 1) * KB],
            rhs=qt_sb[0:64, pr, j * QB + qs : (j + 1) * QB],
            start=True,
            stop=True,
            tile_position=(0, 0),
        )
        nc.tensor.matmul(
            s_ps[:, 1, qs:QB],
            lhsT=kt_sb[64:128, pr, i * KB : (i + 1) * KB],
            rhs=qt_sb[64:128, pr, j * QB + qs : (j + 1) * QB],
            start=True,
            stop=True,
            tile_position=(64, 0),
        )

    def mm2_pair(pr, j, i, n_i, p_sb, qs, ctx_A, ctx_B):
        """BMM2: ctx^T[0:64] += V.T @ P^T ; row 64 accumulates sums."""
        nc.tensor.matmul(
            ctx_A[:, qs:QB],
            lhsT=vp_sb[:, 2 * pr, i, :],
            rhs=p_sb[:, 0, qs:QB],
            start=(i == 0),
            stop=(i == n_i - 1),
        )
        nc.tensor.matmul(
            ctx_B[:, qs:QB],
            lhsT=vp_sb[:, 2 * pr + 1, i, :],
            rhs=p_sb[:, 1, qs:QB],
            start=(i == 0),
            stop=(i == n_i - 1),
        )

    # Flat slot list across all (pair, j, i) so the software pipeline runs
    # through j/pair boundaries without a bubble. j descending (longest
    # i-loops first, so the kernel tail is short).
    slots = []
    for pr in range(NPAIR):
        for j in range(NQB - 1, -1, -1):
            n_i = 4 * j + 4  # causal: k blocks 0 .. 4j+3
            for i in range(n_i):
                slots.append((pr, j, i, n_i))

    pending = []  # queue of (pr, j, i, n_i, p_sb, qs, ctx_A, ctx_B)
    ctxs = {}

    def drain_one():
        ent = pending.pop(0)
        mm2_pair(*ent)
        if ent[2] == ent[3] - 1:  # finished a (pair, j): epilogue
            ppr, pj = ent[0], ent[1]
            # Split the two PSUM->SBUF copies across DVE and the slack
            # ScalarE so neither blocks its engine's pipeline at the
            # j boundary.
            last = ppr == NPAIR - 1 and pj == 0
            for g, ctx, eng in (
                (2 * ppr, ent[6], "v"),
                (2 * ppr + 1, ent[7], "v" if last else "s"),
            ):
                ctxn = ep.tile([D + 1, QB], F32, tag="ctxn")
                if eng == "v":
                    nc.vector.tensor_copy(ctxn, ctx)
                else:
                    nc.scalar.copy(ctxn, ctx)
                nc.sync.dma_start(
                    out=out[g, :, pj * QB : (pj + 1) * QB], in_=ctxn
                )

    n_offd = 0
    for pr, j, i, n_i in slots:
        if i == 0:
            ctxs[(pr, j)] = (
                psum_c.tile([D + 1, QB], F32, tag="ctxA", name="ctxA"),
                psum_c.tile([D + 1, QB], F32, tag="ctxB", name="ctxB"),
            )
        ctx_A, ctx_B = ctxs[(pr, j)]
        t = i - 4 * j
        qs = max(0, 128 * t)  # q start within the 512 block
        s_ps = psum_s.tile([128, 2, QB], F32, tag="s")
        mm1_pair(pr, j, i, s_ps)
        # Lag-1 software pipeline; at each j's first two slots skip the
        # drain so the previous j's diagonal-tail BMM2s (whose P arrives
        # late via the exp+mask chain) drop off the critical path.
        if i >= 2:
            while len(pending) > 1:
                drain_one()
        if t < 0 and (n_offd := n_offd + 1) % 2:
            # Offload this slot's exp to DVE via the int16-Schraudolph
            # trick: bf16(exp(s/8)) == bitcast(int16(s * SCH_A + SCH_B)).
            p_i16 = pp.tile([128, 2, QB], I16, tag="p")
            nc.vector.tensor_scalar(
                out=p_i16[:, :, qs:QB],
                in0=s_ps[:, :, qs:QB],
                scalar1=SCH_A,
                scalar2=SCH_B,
                op0=mybir.AluOpType.mult,
                op1=mybir.AluOpType.add,
            )
            p_sb = p_i16.bitcast(BF16)
        else:
            p_sb = pp.tile([128, 2, QB], BF16, tag="p")
            nc.scalar.activation(
                p_sb[:, :, qs:QB],
                s_ps[:, :, qs:QB],
                mybir.ActivationFunctionType.Exp,
                scale=1.0 / NORM,
            )
            if t >= 0:
                # Diagonal sub-block: zero the strict upper triangle
                # (q < k) of P for both heads, post-exp, on GpSimd.
                nc.gpsimd.affine_select(
                    out=p_sb[:, :, qs : qs + 128],
                    in_=p_sb[:, :, qs : qs + 128],
                    compare_op=mybir.AluOpType.is_ge,
                    fill=0.0,
                    base=0,
                    pattern=[[0, 2], [1, 128]],  # iota over q, ignore head
                    channel_multiplier=-1,  # -k per partition
                )
        pending.append((pr, j, i, n_i, p_sb, qs, ctx_A, ctx_B))
    while pending:
        drain_one()


def _build_nc():
    nc = bacc.Bacc(
        "TRN2", target_bir_lowering=False, debug=False, num_devices=NCORES
    )
    with tile.TileContext(nc) as tc, ExitStack() as ctx_stack:
        build_attention(nc, tc, ctx_stack)
    nc.compile()
    return nc


_NC_CACHE = {}


def get_nc():
    if "nc" not in _NC_CACHE:
        _NC_CACHE["nc"] = _build_nc()
    return _NC_CACHE["nc"]


def shard_inputs(query_layer, key_layer, value_layer):
    """Full [SQ, B, H, D] fp32 inputs -> list of 8 per-core input dicts."""
    import ml_dtypes

    bf16 = np.dtype(ml_dtypes.bfloat16)
    q = np.asarray(query_layer, dtype=np.float32)
    k = np.asarray(key_layer, dtype=np.float32)
    v = np.asarray(value_layer, dtype=np.float32)
    # [SQ, B, H, D] -> [B*H, D, SQ] (d-major) for Q/K; [B*H, SQ, D] for V.
    qt = np.ascontiguousarray(q.transpose(1, 2, 3, 0).reshape(B * H, D, SQ))
    kt = np.ascontiguousarray(k.transpose(1, 2, 3, 0).reshape(B * H, D, SQ))
    vn = v.transpose(1, 2, 0, 3).reshape(B * H, SQ, D)
    vn = np.concatenate([vn, np.ones((B * H, SQ, 1), np.float32)], axis=2)
    qt = qt.astype(bf16)
    kt = kt.astype(bf16)
    vn = np.ascontiguousarray(vn.astype(bf16))
    in_maps = []
    for c in range(NCORES):
        sl = slice(HPC * c, HPC * (c + 1))
        # Swizzle v to the device SBUF layout [128, HPC, NKB, 65].
        vc = vn[sl].reshape(HPC, NKB, 128, D + 1).transpose(2, 0, 1, 3)
        in_maps.append(
            {
                "qt": np.ascontiguousarray(qt[sl].reshape(HPC * D, SQ)),
                "kt": np.ascontiguousarray(kt[sl].reshape(HPC * D, SQ)),
                "v": np.ascontiguousarray(vc.reshape(128, HPC * NKB * (D + 1))),
            }
        )
    return in_maps


def gather_outputs(results):
    """8 per-core {'out': [4, 65, 2048]} -> full [SQ, B, H*D] fp32."""
    raw = np.stack([np.asarray(results[c]["out"], dtype=np.float32) for c in range(NCORES)])
    raw = raw.reshape(B * H, D + 1, SQ)
    ctx_t = raw[:, :D, :] / raw[:, D : D + 1, :]  # normalize by denominator
    full = ctx_t.transpose(2, 0, 1).reshape(SQ, B, H * D)
    return np.ascontiguousarray(full.astype(np.float32))


def run_on_device(in_maps, trace=False):
    from concourse.bass_utils import run_bass_kernel_spmd

    nc = get_nc()
    res = run_bass_kernel_spmd(
        nc, in_maps, core_ids=list(range(NCORES)), trace=trace
    )
    return res


def kernel(query_layer, key_layer, value_layer, attention_mask=None):
    in_maps = shard_inputs(query_layer, key_layer, value_layer)
    res = run_on_device(in_maps, trace=False)
    return gather_outputs(res.results)
